# revision 2
# baseline (speedup 1.0000x reference)
"""DANet-style Dual Attention Module (channel + position attention) on 8 TRN2 cores.

Sharding: data-parallel over batch (4) x position-halves (2) = 8 cores.
Each core computes, for its (batch b, n-half h):
    y = 2*x + beta*feat_e + alpha*feat_p   restricted to columns of its half.
Inputs are pre-rolled on the host so every core runs an identical program
(its half is always columns 0:NH of its private x copy).

Channel-attention scores (x @ x.T over all N=4096 positions) are computed in a
3-pass bf16 hi/lo decomposition (hi*hi + hi*lo + lo*hi) so the transposed
operand can be produced with the 2-byte DMA xbar transpose; scores for the
position attention (fb/fc projections) are computed in fp32.  Value-side
matmuls run in bf16.  The 2*x term is computed exactly on the vector engine
from the fp32 input.
"""

import sys

sys.path.insert(0, "/opt/trn_rl_repo")

from contextlib import ExitStack

import numpy as np
import ml_dtypes

import concourse.bass as bass
import concourse.tile as tile
from concourse import bacc, mybir
from concourse.bass_utils import run_bass_kernel_spmd

F32 = mybir.dt.float32
BF16 = mybir.dt.bfloat16
AX = mybir.AxisListType
ALU = mybir.AluOpType
ACTF = mybir.ActivationFunctionType
BF = ml_dtypes.bfloat16

B, C, H, W = 4, 512, 64, 64
N = H * W            # 4096
NH = N // 2          # per-core position half
CP = C // 8          # 64 projection channels
N_CORES = 8


def _build_program(tc, ins, y_ap, C=C, N=N, NH=NH, CP=CP, lolo_pass=False):
    nc = tc.nc
    KT = C // 128          # channel k-tiles
    MT = N // 128          # position tiles (keys)
    CT = C // 128          # output channel tiles
    CHUNK = 512
    NCH = NH // CHUNK      # output column chunks

    x_f = ins["x"]

    ctx = ExitStack()
    sb = ctx.enter_context(tc.tile_pool(name="sb", bufs=1))
    ps = ctx.enter_context(tc.tile_pool(name="ps", bufs=1, space="PSUM"))

    def pst(shape, dtype=F32, name="pst"):
        return ps.tile(shape, dtype, tag="ps", bufs=8, name=name)

    # ---------------- constants / weights ----------------
    wcT = sb.tile([128, KT * CP], F32, name="wcT")
    nc.sync.dma_start(wcT[:].rearrange("p (kt m) -> p kt m", kt=KT),
                      ins["wcT"].rearrange("(kt p) m -> p kt m", p=128))
    wbT = sb.tile([128, KT * CP], F32, name="wbT")
    nc.sync.dma_start(wbT[:].rearrange("p (kt m) -> p kt m", kt=KT),
                      ins["wbT"].rearrange("(kt p) m -> p kt m", p=128))
    wdT = sb.tile([128, KT * C], BF16, name="wdT")
    nc.sync.dma_start(wdT[:].rearrange("p (kt m) -> p kt m", kt=KT),
                      ins["wdT"].rearrange("(kt p) m -> p kt m", p=128))
    bc_t = sb.tile([128, 1], F32, name="bc_t")
    nc.sync.dma_start(bc_t[:], ins["bc"])
    bb_t = sb.tile([128, 1], F32, name="bb_t")
    nc.sync.dma_start(bb_t[:], ins["bb"])
    bdrow = sb.tile([1, C], BF16, name="bdrow")
    nc.sync.dma_start(bdrow[:], ins["bdrow"])
    beta_t = sb.tile([128, 1], F32, name="beta_t")
    nc.sync.dma_start(beta_t[:], ins["beta"])
    alpha_t = sb.tile([1, 1], F32, name="alpha_t")
    nc.sync.dma_start(alpha_t[:], ins["alpha"])
    ident = sb.tile([128, 128], BF16, name="ident")
    nc.sync.dma_start(ident[:], ins["ident"])
    ones128 = sb.tile([128, 1], BF16, name="ones128")
    nc.sync.dma_start(ones128[:], ins["ones128"])
    onesrow_bf = sb.tile([1, 128], BF16, name="onesrow_bf")
    nc.sync.dma_start(onesrow_bf[:], ins["onesrow_bf"])
    onesrow_f32 = sb.tile([1, 128], F32, name="onesrow_f32")
    nc.sync.dma_start(onesrow_f32[:], ins["onesrow_f32"])

    x3 = x_f.rearrange("(kt p) n -> p kt n", p=128)  # [128, KT, N] DRAM view

    # ---------------- stage 1: fc (full) and fb (first NH cols) ----------------
    fc_t = sb.tile([64, N], F32, name="fc_t")
    fb_t = sb.tile([64, NH], F32, name="fb_t")
    for ch in range(N // CHUNK):
        xs = sb.tile([128, KT * CHUNK], F32, tag="xs", bufs=2, name="xs")
        nc.sync.dma_start(xs[:].rearrange("p (kt n) -> p kt n", kt=KT),
                          x3[:, :, ch * CHUNK:(ch + 1) * CHUNK])
        ps_fc = pst([64, CHUNK], name="ps_fc")
        for kt in range(KT):
            nc.tensor.matmul(ps_fc[:], wcT[:, kt * CP:(kt + 1) * CP],
                             xs[:, kt * CHUNK:(kt + 1) * CHUNK],
                             start=(kt == 0), stop=(kt == KT - 1))
        nc.scalar.add(fc_t[:, ch * CHUNK:(ch + 1) * CHUNK], ps_fc[:], bc_t[0:64, :])
        if ch < NH // CHUNK:
            ps_fb = pst([64, CHUNK], name="ps_fb")
            for kt in range(KT):
                nc.tensor.matmul(ps_fb[:], wbT[:, kt * CP:(kt + 1) * CP],
                                 xs[:, kt * CHUNK:(kt + 1) * CHUNK],
                                 start=(kt == 0), stop=(kt == KT - 1))
            nc.scalar.add(fb_t[:, ch * CHUNK:(ch + 1) * CHUNK], ps_fb[:], bb_t[0:64, :])

    # ---------------- stage 2: fdT [m, c] = (wd @ x + bd).T ----------------
    fdT = sb.tile([128, MT * C], BF16, name="fdT")
    for mt in range(MT):
        fdl = sb.tile([128, KT * 128], BF16, tag="fdl", bufs=3, name="fdl")
        nc.gpsimd.dma_start(fdl[:].rearrange("p (kt n) -> p kt n", kt=KT),
                            x3[:, :, mt * 128:(mt + 1) * 128])
        ps_d = pst([128, C], name="ps_d")
        for kt in range(KT):
            nc.tensor.matmul(ps_d[:], fdl[:, kt * 128:(kt + 1) * 128],
                             wdT[:, kt * C:(kt + 1) * C],
                             start=(kt == 0), stop=False)
        nc.tensor.matmul(ps_d[:], onesrow_bf[:], bdrow[:], start=False, stop=True)
        nc.scalar.copy(fdT[:, mt * C:(mt + 1) * C], ps_d[:])

    # ---------------- stage 3: channel attention scores (hi/lo passes) ----------------
    ps_att = [pst([128, C], name=f"ps_att{ct}") for ct in range(CT)]
    xhi3, xlo3 = ins["xhi"], ins["xlo"]
    for nt in range(MT):
        hiT = sb.tile([128, C], BF16, tag="hiT", bufs=4, name="hiT")
        nc.sync.dma_start_transpose(hiT[:], xhi3[:, nt * 128:(nt + 1) * 128])
        loT = sb.tile([128, C], BF16, tag="loT", bufs=4, name="loT")
        nc.sync.dma_start_transpose(loT[:], xlo3[:, nt * 128:(nt + 1) * 128])
        for ct in range(CT):
            cs = slice(ct * 128, (ct + 1) * 128)
            nc.tensor.matmul(ps_att[ct][:], hiT[:, cs], hiT[:, 0:C],
                             start=(nt == 0), stop=False)
            nc.tensor.matmul(ps_att[ct][:], hiT[:, cs], loT[:, 0:C],
                             start=False, stop=False)
            last = (nt == MT - 1) and not lolo_pass
            nc.tensor.matmul(ps_att[ct][:], loT[:, cs], hiT[:, 0:C],
                             start=False, stop=last)
            if lolo_pass:
                nc.tensor.matmul(ps_att[ct][:], loT[:, cs], loT[:, 0:C],
                                 start=False, stop=(nt == MT - 1))

    # ---------------- stage 3.5: channel softmax + transpose ----------------
    # softmax(rowmax - att) == exp(rowmin - att) / sum(exp(rowmin - att))
    rmin = sb.tile([128, CT], F32, name="rmin")
    attS = sb.tile([128, CT], F32, name="attS")
    recipc = sb.tile([128, CT], F32, name="recipc")
    attcT = sb.tile([128, KT * C], BF16, name="attcT")
    attc_tiles = []
    for ct in range(CT):
        nc.vector.tensor_reduce(rmin[:, ct:ct + 1], ps_att[ct][:], axis=AX.X, op=ALU.min)
        atte = sb.tile([128, C], F32, tag="atte", bufs=2, name="atte")
        nc.scalar.activation(atte[:], ps_att[ct][:], ACTF.Exp,
                             bias=rmin[:, ct:ct + 1], scale=-1.0,
                             accum_out=attS[:, ct:ct + 1])
        nc.vector.reciprocal(recipc[:, ct:ct + 1], attS[:, ct:ct + 1])
        attc = sb.tile([128, C], BF16, tag="attc", bufs=2, name="attc")
        nc.vector.tensor_scalar(attc[:], atte[:], recipc[:, ct:ct + 1], beta_t[:, 0:1],
                                op0=ALU.mult, op1=ALU.mult)
        attc_tiles.append(attc)
    for ct in range(CT):
        for dt in range(CT):
            ps_t = ps.tile([128, 128], BF16, tag="ps", bufs=8, name="ps_t")
            nc.tensor.transpose(ps_t[:], attc_tiles[ct][:, dt * 128:(dt + 1) * 128], ident[:])
            nc.scalar.copy(attcT[:, dt * C + ct * 128: dt * C + (ct + 1) * 128], ps_t[:])

    # ---------------- stage 4: position attention + combine, per 512-col chunk ----------------
    for ch in range(NCH):
        ncs = slice(ch * CHUNK, (ch + 1) * CHUNK)
        slab = sb.tile([128, MT * CHUNK], BF16, tag="slab", bufs=2, name="slab")
        ps_S = pst([1, CHUNK], name="ps_S")
        for mt in range(MT):
            ps_s = pst([128, CHUNK], name="ps_s")
            nc.tensor.matmul(ps_s[:], fc_t[0:64, mt * 128:(mt + 1) * 128],
                             fb_t[0:64, ncs], start=True, stop=True)
            nc.scalar.activation(slab[:, mt * CHUNK:(mt + 1) * CHUNK], ps_s[:], ACTF.Exp)
            nc.tensor.matmul(ps_S[:], ones128[:], slab[:, mt * CHUNK:(mt + 1) * CHUNK],
                             start=(mt == 0), stop=(mt == MT - 1))
        recipS = sb.tile([1, CHUNK], F32, tag="recipS", bufs=2, name="recipS")
        nc.vector.reciprocal(recipS[:], ps_S[:])
        recipSa = sb.tile([1, CHUNK], F32, tag="recipSa", bufs=2, name="recipSa")
        nc.vector.tensor_scalar(recipSa[:], recipS[:], alpha_t[0:1, 0:1], None, op0=ALU.mult)
        ps_bc = pst([128, CHUNK], name="ps_bc")
        nc.tensor.matmul(ps_bc[:], onesrow_f32[:], recipSa[:], start=True, stop=True)
        bcast = sb.tile([128, CHUNK], F32, tag="bcast", bufs=2, name="bcast")
        nc.scalar.copy(bcast[:], ps_bc[:])

        fer = sb.tile([128, KT * CHUNK], BF16, tag="fer", bufs=2, name="fer")
        nc.gpsimd.dma_start(fer[:].rearrange("p (kt n) -> p kt n", kt=KT),
                            x3[:, :, ncs])
        for ct in range(CT):
            ps_A = pst([128, CHUNK], name="ps_A")
            for mt in range(MT):
                nc.tensor.matmul(ps_A[:], fdT[:, mt * C + ct * 128: mt * C + (ct + 1) * 128],
                                 slab[:, mt * CHUNK:(mt + 1) * CHUNK],
                                 start=(mt == 0), stop=(mt == MT - 1))
            ps_C = pst([128, CHUNK], name="ps_C")
            for dt in range(KT):
                nc.tensor.matmul(ps_C[:], attcT[:, dt * C + ct * 128: dt * C + (ct + 1) * 128],
                                 fer[:, dt * CHUNK:(dt + 1) * CHUNK],
                                 start=(dt == 0), stop=(dt == KT - 1))
            x2 = sb.tile([128, CHUNK], F32, tag="x2", bufs=2, name="x2")
            nc.sync.dma_start(x2[:], x_f[ct * 128:(ct + 1) * 128, ncs])
            t1 = sb.tile([128, CHUNK], F32, tag="t1", bufs=2, name="t1")
            nc.vector.tensor_mul(t1[:], ps_A[:], bcast[:])
            t2 = sb.tile([128, CHUNK], F32, tag="t2", bufs=2, name="t2")
            nc.vector.scalar_tensor_tensor(t2[:], x2[:], 2.0, ps_C[:],
                                           op0=ALU.mult, op1=ALU.add)
            outt = sb.tile([128, CHUNK], F32, tag="outt", bufs=3, name="outt")
            nc.vector.tensor_add(outt[:], t1[:], t2[:])
            nc.sync.dma_start(y_ap[ct * 128:(ct + 1) * 128, ncs], outt[:])


_CACHE = {}

_INPUT_SPECS = [
    ("x", [C, N], F32),
    ("xhi", [C, N], BF16),
    ("xlo", [C, N], BF16),
    ("wcT", [C, CP], F32),
    ("wbT", [C, CP], F32),
    ("wdT", [C, C], BF16),
    ("bc", [128, 1], F32),
    ("bb", [128, 1], F32),
    ("bdrow", [1, C], BF16),
    ("beta", [128, 1], F32),
    ("alpha", [1, 1], F32),
    ("ident", [128, 128], BF16),
    ("ones128", [128, 1], BF16),
    ("onesrow_bf", [1, 128], BF16),
    ("onesrow_f32", [1, 128], F32),
]


def get_compiled():
    if "nc" in _CACHE:
        return _CACHE["nc"]
    nc = bacc.Bacc("TRN2", target_bir_lowering=False, debug=False,
                   num_devices=N_CORES)
    ins = {}
    for name, shape, dt in _INPUT_SPECS:
        ins[name] = nc.dram_tensor(name, shape, dt, kind="ExternalInput").ap()
    y_ap = nc.dram_tensor("y", [C, NH], F32, kind="ExternalOutput").ap()
    with tile.TileContext(nc) as tc:
        _build_program(tc, ins, y_ap)
    nc.compile()
    _CACHE["nc"] = nc
    return nc


def make_in_maps(x, wb, bb, wc, bc, wd, bd, alpha, beta):
    """Build the 8 per-core input maps from the full problem inputs."""
    xb = np.ascontiguousarray(np.asarray(x, dtype=np.float32)).reshape(B, C, N)
    wb = np.asarray(wb, dtype=np.float32)
    wc = np.asarray(wc, dtype=np.float32)
    wd = np.asarray(wd, dtype=np.float32)
    bb_ = np.asarray(bb, dtype=np.float32).reshape(CP)
    bc_ = np.asarray(bc, dtype=np.float32).reshape(CP)
    bd_ = np.asarray(bd, dtype=np.float32).reshape(C)
    alpha_ = float(np.asarray(alpha).reshape(-1)[0])
    beta_ = float(np.asarray(beta).reshape(-1)[0])

    bc128 = np.zeros((128, 1), np.float32); bc128[:CP, 0] = bc_
    bb128 = np.zeros((128, 1), np.float32); bb128[:CP, 0] = bb_
    shared = {
        "wcT": np.ascontiguousarray(wc.T, dtype=np.float32),
        "wbT": np.ascontiguousarray(wb.T, dtype=np.float32),
        "wdT": np.ascontiguousarray(wd.T).astype(BF),
        "bc": bc128,
        "bb": bb128,
        "bdrow": bd_.reshape(1, C).astype(BF),
        "beta": np.full((128, 1), beta_, np.float32),
        "alpha": np.full((1, 1), alpha_, np.float32),
        "ident": np.eye(128, dtype=BF),
        "ones128": np.ones((128, 1), BF),
        "onesrow_bf": np.ones((1, 128), BF),
        "onesrow_f32": np.ones((1, 128), np.float32),
    }
    in_maps = []
    for core in range(N_CORES):
        b, h = core // 2, core % 2
        xc = xb[b] if h == 0 else np.ascontiguousarray(np.roll(xb[b], -NH, axis=1))
        # hi/lo split of the *unrolled* batch image: the channel-attention
        # score sums over all positions, so position order is irrelevant.
        key = ("hilo", b)
        if key not in _CACHE:
            xhi = xb[b].astype(BF)
            xlo = (xb[b] - xhi.astype(np.float32)).astype(BF)
            _CACHE[key] = (xhi, xlo)
        xhi, xlo = _CACHE[key]
        in_maps.append({"x": xc, "xhi": xhi, "xlo": xlo, **shared})
    return in_maps


def assemble_output(results):
    out = np.empty((B, C, N), np.float32)
    for core in range(N_CORES):
        b, h = core // 2, core % 2
        out[b][:, h * NH:(h + 1) * NH] = results[core]["y"]
    return out.reshape(B, C, H, W)


def kernel(x, wb, bb, wc, bc, wd, bd, alpha, beta):
    nc = get_compiled()
    in_maps = make_in_maps(x, wb, bb, wc, bc, wd, bd, alpha, beta)
    res = run_bass_kernel_spmd(nc, in_maps, list(range(N_CORES)))
    for key in [("hilo", b) for b in range(B)]:
        _CACHE.pop(key, None)
    return assemble_output(res.results)


# revision 4
# speedup vs baseline: 1.0601x; 1.0601x over previous
"""DANet-style Dual Attention Module (channel + position attention) on 8 TRN2 cores.

Sharding: data-parallel over batch (4) x position-halves (2) = 8 cores.
Each core computes, for its (batch b, n-half h):
    y = 2*x + beta*feat_e + alpha*feat_p   restricted to columns of its half.
Inputs are pre-rolled on the host so every core runs an identical program
(its half is always columns 0:NH of its private x copy).

Channel-attention scores (x @ x.T over all N=4096 positions) are computed in a
3-pass bf16 hi/lo decomposition (hi*hi + hi*lo + lo*hi) so the transposed
operand can be produced with the 2-byte DMA xbar transpose; scores for the
position attention (fb/fc projections) are computed in fp32.  Value-side
matmuls run in bf16.  The 2*x term is computed exactly on the vector engine
from the fp32 input.
"""

import sys

sys.path.insert(0, "/opt/trn_rl_repo")

from contextlib import ExitStack

import numpy as np
import ml_dtypes

import concourse.bass as bass
import concourse.tile as tile
from concourse import bacc, mybir
from concourse.bass_utils import run_bass_kernel_spmd

F32 = mybir.dt.float32
BF16 = mybir.dt.bfloat16
AX = mybir.AxisListType
ALU = mybir.AluOpType
ACTF = mybir.ActivationFunctionType
BF = ml_dtypes.bfloat16

B, C, H, W = 4, 512, 64, 64
N = H * W            # 4096
NH = N // 2          # per-core position half
CP = C // 8          # 64 projection channels
N_CORES = 8


def _build_program(tc, ins, y_ap, C=C, N=N, NH=NH, CP=CP, lolo_pass=False):
    nc = tc.nc
    KT = C // 128          # channel k-tiles
    MT = N // 128          # position tiles (keys)
    CT = C // 128          # output channel tiles
    CHUNK = 512
    NCH = NH // CHUNK      # output column chunks

    x_f = ins["x"]

    ctx = ExitStack()
    sb = ctx.enter_context(tc.tile_pool(name="sb", bufs=1))
    ps = ctx.enter_context(tc.tile_pool(name="ps", bufs=1, space="PSUM"))

    def pst(shape, dtype=F32, name="pst"):
        return ps.tile(shape, dtype, tag="ps", bufs=8, name=name)

    # ---------------- constants / weights ----------------
    wcT = sb.tile([128, KT * CP], F32, name="wcT")
    nc.sync.dma_start(wcT[:].rearrange("p (kt m) -> p kt m", kt=KT),
                      ins["wcT"].rearrange("(kt p) m -> p kt m", p=128))
    wbT = sb.tile([128, KT * CP], F32, name="wbT")
    nc.sync.dma_start(wbT[:].rearrange("p (kt m) -> p kt m", kt=KT),
                      ins["wbT"].rearrange("(kt p) m -> p kt m", p=128))
    wdT = sb.tile([128, KT * C], BF16, name="wdT")
    nc.sync.dma_start(wdT[:].rearrange("p (kt m) -> p kt m", kt=KT),
                      ins["wdT"].rearrange("(kt p) m -> p kt m", p=128))
    bc_t = sb.tile([128, 1], F32, name="bc_t")
    nc.sync.dma_start(bc_t[:], ins["bc"])
    bb_t = sb.tile([128, 1], F32, name="bb_t")
    nc.sync.dma_start(bb_t[:], ins["bb"])
    bdrow = sb.tile([1, C], BF16, name="bdrow")
    nc.sync.dma_start(bdrow[:], ins["bdrow"])
    beta_t = sb.tile([128, 1], F32, name="beta_t")
    nc.sync.dma_start(beta_t[:], ins["beta"])
    alpha_t = sb.tile([1, 1], F32, name="alpha_t")
    nc.sync.dma_start(alpha_t[:], ins["alpha"])
    ident = sb.tile([128, 128], BF16, name="ident")
    nc.sync.dma_start(ident[:], ins["ident"])
    ones128 = sb.tile([128, 1], BF16, name="ones128")
    nc.sync.dma_start(ones128[:], ins["ones128"])
    onesrow_bf = sb.tile([1, 128], BF16, name="onesrow_bf")
    nc.sync.dma_start(onesrow_bf[:], ins["onesrow_bf"])
    onesrow_f32 = sb.tile([1, 128], F32, name="onesrow_f32")
    nc.sync.dma_start(onesrow_f32[:], ins["onesrow_f32"])

    x3 = x_f.rearrange("(kt p) n -> p kt n", p=128)  # [128, KT, N] DRAM view

    # ---------------- stage 1: fc (full), fb (first NH cols), fdT ----------------
    fc_t = sb.tile([64, N], F32, name="fc_t")
    fb_t = sb.tile([64, NH], F32, name="fb_t")
    fdT = sb.tile([128, MT * C], BF16, name="fdT")
    for ch in range(N // CHUNK):
        xs = sb.tile([128, KT * CHUNK], F32, tag="xs", bufs=2, name="xs")
        nc.sync.dma_start(xs[:].rearrange("p (kt n) -> p kt n", kt=KT),
                          x3[:, :, ch * CHUNK:(ch + 1) * CHUNK])
        xsb = sb.tile([128, KT * CHUNK], BF16, tag="xsb", bufs=2, name="xsb")
        nc.vector.tensor_copy(xsb[:], xs[:])
        ps_fc = pst([64, CHUNK], name="ps_fc")
        for kt in range(KT):
            nc.tensor.matmul(ps_fc[:], wcT[:, kt * CP:(kt + 1) * CP],
                             xs[:, kt * CHUNK:(kt + 1) * CHUNK],
                             start=(kt == 0), stop=(kt == KT - 1))
        nc.scalar.add(fc_t[:, ch * CHUNK:(ch + 1) * CHUNK], ps_fc[:], bc_t[0:64, :])
        if ch < NH // CHUNK:
            ps_fb = pst([64, CHUNK], name="ps_fb")
            for kt in range(KT):
                nc.tensor.matmul(ps_fb[:], wbT[:, kt * CP:(kt + 1) * CP],
                                 xs[:, kt * CHUNK:(kt + 1) * CHUNK],
                                 start=(kt == 0), stop=(kt == KT - 1))
            nc.scalar.add(fb_t[:, ch * CHUNK:(ch + 1) * CHUNK], ps_fb[:], bb_t[0:64, :])
        for j in range(CHUNK // 128):
            mt = ch * (CHUNK // 128) + j
            ps_d = pst([128, C], name="ps_d")
            for kt in range(KT):
                nc.tensor.matmul(ps_d[:], xsb[:, kt * CHUNK + j * 128: kt * CHUNK + (j + 1) * 128],
                                 wdT[:, kt * C:(kt + 1) * C],
                                 start=(kt == 0), stop=False)
            nc.tensor.matmul(ps_d[:], onesrow_bf[:], bdrow[:], start=False, stop=True)
            nc.scalar.copy(fdT[:, mt * C:(mt + 1) * C], ps_d[:])

    # ---------------- stage 3: channel attention scores (hi/lo passes) ----------------
    ps_att = [pst([128, C], name=f"ps_att{ct}") for ct in range(CT)]
    xhi3, xlo3 = ins["xhi"], ins["xlo"]
    for nt in range(MT):
        hiT = sb.tile([128, C], BF16, tag="hiT", bufs=3, name="hiT")
        nc.sync.dma_start_transpose(hiT[:], xhi3[:, nt * 128:(nt + 1) * 128])
        loT = sb.tile([128, C], BF16, tag="loT", bufs=3, name="loT")
        nc.sync.dma_start_transpose(loT[:], xlo3[:, nt * 128:(nt + 1) * 128])
        for ct in range(CT):
            cs = slice(ct * 128, (ct + 1) * 128)
            nc.tensor.matmul(ps_att[ct][:], hiT[:, cs], hiT[:, 0:C],
                             start=(nt == 0), stop=False)
            nc.tensor.matmul(ps_att[ct][:], hiT[:, cs], loT[:, 0:C],
                             start=False, stop=False)
            last = (nt == MT - 1) and not lolo_pass
            nc.tensor.matmul(ps_att[ct][:], loT[:, cs], hiT[:, 0:C],
                             start=False, stop=last)
            if lolo_pass:
                nc.tensor.matmul(ps_att[ct][:], loT[:, cs], loT[:, 0:C],
                                 start=False, stop=(nt == MT - 1))

    # ---------------- stage 3.5: channel softmax + transpose ----------------
    # softmax(rowmax - att) == exp(rowmin - att) / sum(exp(rowmin - att))
    rmin = sb.tile([128, CT], F32, name="rmin")
    attS = sb.tile([128, CT], F32, name="attS")
    recipc = sb.tile([128, CT], F32, name="recipc")
    attcT = sb.tile([128, KT * C], BF16, name="attcT")
    attc_tiles = []
    for ct in range(CT):
        nc.vector.tensor_reduce(rmin[:, ct:ct + 1], ps_att[ct][:], axis=AX.X, op=ALU.min)
        atte = sb.tile([128, C], F32, tag="atte", bufs=2, name="atte")
        nc.scalar.activation(atte[:], ps_att[ct][:], ACTF.Exp,
                             bias=rmin[:, ct:ct + 1], scale=-1.0,
                             accum_out=attS[:, ct:ct + 1])
        nc.vector.reciprocal(recipc[:, ct:ct + 1], attS[:, ct:ct + 1])
        attc = sb.tile([128, C], BF16, tag="attc", bufs=2, name="attc")
        nc.vector.tensor_scalar(attc[:], atte[:], recipc[:, ct:ct + 1], beta_t[:, 0:1],
                                op0=ALU.mult, op1=ALU.mult)
        attc_tiles.append(attc)
    for ct in range(CT):
        for dt in range(CT):
            ps_t = ps.tile([128, 128], BF16, tag="ps", bufs=8, name="ps_t")
            nc.tensor.transpose(ps_t[:], attc_tiles[ct][:, dt * 128:(dt + 1) * 128], ident[:])
            nc.scalar.copy(attcT[:, dt * C + ct * 128: dt * C + (ct + 1) * 128], ps_t[:])

    # ---------------- stage 4: position attention + combine, per 512-col chunk ----------------
    for ch in range(NCH):
        ncs = slice(ch * CHUNK, (ch + 1) * CHUNK)
        slab = sb.tile([128, MT * CHUNK], BF16, tag="slab", bufs=2, name="slab")
        ps_S = pst([1, CHUNK], name="ps_S")
        for mt in range(MT):
            ps_s = pst([128, CHUNK], name="ps_s")
            nc.tensor.matmul(ps_s[:], fc_t[0:64, mt * 128:(mt + 1) * 128],
                             fb_t[0:64, ncs], start=True, stop=True)
            nc.scalar.activation(slab[:, mt * CHUNK:(mt + 1) * CHUNK], ps_s[:], ACTF.Exp)
            nc.tensor.matmul(ps_S[:], ones128[:], slab[:, mt * CHUNK:(mt + 1) * CHUNK],
                             start=(mt == 0), stop=(mt == MT - 1))
        recipS = sb.tile([1, CHUNK], F32, tag="recipS", bufs=1, name="recipS")
        nc.vector.reciprocal(recipS[:], ps_S[:])
        recipSa = sb.tile([1, CHUNK], F32, tag="recipSa", bufs=1, name="recipSa")
        nc.vector.tensor_scalar(recipSa[:], recipS[:], alpha_t[0:1, 0:1], None, op0=ALU.mult)
        ps_bc = pst([128, CHUNK], name="ps_bc")
        nc.tensor.matmul(ps_bc[:], onesrow_f32[:], recipSa[:], start=True, stop=True)
        bcast = sb.tile([128, CHUNK], F32, tag="bcast", bufs=2, name="bcast")
        nc.scalar.copy(bcast[:], ps_bc[:])

        x2_tiles, x2b_tiles = [], []
        for dt in range(CT):
            x2 = sb.tile([128, CHUNK], F32, tag="x2", bufs=5, name="x2")
            nc.sync.dma_start(x2[:], x_f[dt * 128:(dt + 1) * 128, ncs])
            x2b = sb.tile([128, CHUNK], BF16, tag="x2b", bufs=5, name="x2b")
            nc.vector.tensor_copy(x2b[:], x2[:])
            x2_tiles.append(x2)
            x2b_tiles.append(x2b)
        for ct in range(CT):
            ps_A = pst([128, CHUNK], name="ps_A")
            for mt in range(MT):
                nc.tensor.matmul(ps_A[:], fdT[:, mt * C + ct * 128: mt * C + (ct + 1) * 128],
                                 slab[:, mt * CHUNK:(mt + 1) * CHUNK],
                                 start=(mt == 0), stop=(mt == MT - 1))
            ps_C = pst([128, CHUNK], name="ps_C")
            for dt in range(KT):
                nc.tensor.matmul(ps_C[:], attcT[:, dt * C + ct * 128: dt * C + (ct + 1) * 128],
                                 x2b_tiles[dt][:],
                                 start=(dt == 0), stop=(dt == KT - 1))
            t1 = sb.tile([128, CHUNK], F32, tag="t1", bufs=2, name="t1")
            nc.vector.tensor_mul(t1[:], ps_A[:], bcast[:])
            t2 = sb.tile([128, CHUNK], F32, tag="t2", bufs=2, name="t2")
            nc.vector.scalar_tensor_tensor(t2[:], x2_tiles[ct][:], 2.0, ps_C[:],
                                           op0=ALU.mult, op1=ALU.add)
            outt = sb.tile([128, CHUNK], F32, tag="outt", bufs=3, name="outt")
            nc.vector.tensor_add(outt[:], t1[:], t2[:])
            nc.sync.dma_start(y_ap[ct * 128:(ct + 1) * 128, ncs], outt[:])

    ctx.close()


_CACHE = {}

_INPUT_SPECS = [
    ("x", [C, N], F32),
    ("xhi", [C, N], BF16),
    ("xlo", [C, N], BF16),
    ("wcT", [C, CP], F32),
    ("wbT", [C, CP], F32),
    ("wdT", [C, C], BF16),
    ("bc", [128, 1], F32),
    ("bb", [128, 1], F32),
    ("bdrow", [1, C], BF16),
    ("beta", [128, 1], F32),
    ("alpha", [1, 1], F32),
    ("ident", [128, 128], BF16),
    ("ones128", [128, 1], BF16),
    ("onesrow_bf", [1, 128], BF16),
    ("onesrow_f32", [1, 128], F32),
]


def get_compiled():
    if "nc" in _CACHE:
        return _CACHE["nc"]
    nc = bacc.Bacc("TRN2", target_bir_lowering=False, debug=False,
                   num_devices=N_CORES)
    ins = {}
    for name, shape, dt in _INPUT_SPECS:
        ins[name] = nc.dram_tensor(name, shape, dt, kind="ExternalInput").ap()
    y_ap = nc.dram_tensor("y", [C, NH], F32, kind="ExternalOutput").ap()
    with tile.TileContext(nc) as tc:
        _build_program(tc, ins, y_ap)
    nc.compile()
    _CACHE["nc"] = nc
    return nc


def make_in_maps(x, wb, bb, wc, bc, wd, bd, alpha, beta):
    """Build the 8 per-core input maps from the full problem inputs."""
    xb = np.ascontiguousarray(np.asarray(x, dtype=np.float32)).reshape(B, C, N)
    wb = np.asarray(wb, dtype=np.float32)
    wc = np.asarray(wc, dtype=np.float32)
    wd = np.asarray(wd, dtype=np.float32)
    bb_ = np.asarray(bb, dtype=np.float32).reshape(CP)
    bc_ = np.asarray(bc, dtype=np.float32).reshape(CP)
    bd_ = np.asarray(bd, dtype=np.float32).reshape(C)
    alpha_ = float(np.asarray(alpha).reshape(-1)[0])
    beta_ = float(np.asarray(beta).reshape(-1)[0])

    bc128 = np.zeros((128, 1), np.float32); bc128[:CP, 0] = bc_
    bb128 = np.zeros((128, 1), np.float32); bb128[:CP, 0] = bb_
    shared = {
        "wcT": np.ascontiguousarray(wc.T, dtype=np.float32),
        "wbT": np.ascontiguousarray(wb.T, dtype=np.float32),
        "wdT": np.ascontiguousarray(wd.T).astype(BF),
        "bc": bc128,
        "bb": bb128,
        "bdrow": bd_.reshape(1, C).astype(BF),
        "beta": np.full((128, 1), beta_, np.float32),
        "alpha": np.full((1, 1), alpha_, np.float32),
        "ident": np.eye(128, dtype=BF),
        "ones128": np.ones((128, 1), BF),
        "onesrow_bf": np.ones((1, 128), BF),
        "onesrow_f32": np.ones((1, 128), np.float32),
    }
    in_maps = []
    for core in range(N_CORES):
        b, h = core // 2, core % 2
        xc = xb[b] if h == 0 else np.ascontiguousarray(np.roll(xb[b], -NH, axis=1))
        # hi/lo split of the *unrolled* batch image: the channel-attention
        # score sums over all positions, so position order is irrelevant.
        key = ("hilo", b)
        if key not in _CACHE:
            xhi = xb[b].astype(BF)
            xlo = (xb[b] - xhi.astype(np.float32)).astype(BF)
            _CACHE[key] = (xhi, xlo)
        xhi, xlo = _CACHE[key]
        in_maps.append({"x": xc, "xhi": xhi, "xlo": xlo, **shared})
    return in_maps


def assemble_output(results):
    out = np.empty((B, C, N), np.float32)
    for core in range(N_CORES):
        b, h = core // 2, core % 2
        out[b][:, h * NH:(h + 1) * NH] = results[core]["y"]
    return out.reshape(B, C, H, W)


def kernel(x, wb, bb, wc, bc, wd, bd, alpha, beta):
    nc = get_compiled()
    in_maps = make_in_maps(x, wb, bb, wc, bc, wd, bd, alpha, beta)
    res = run_bass_kernel_spmd(nc, in_maps, list(range(N_CORES)))
    for key in [("hilo", b) for b in range(B)]:
        _CACHE.pop(key, None)
    return assemble_output(res.results)


# revision 7
# speedup vs baseline: 105.2674x; 99.2986x over previous
"""DANet-style Dual Attention Module (channel + position attention) on 8 TRN2 cores.

Sharding: data-parallel over batch (4) x position-halves (2) = 8 cores.
Each core computes, for its (batch b, n-half h):
    y = 2*x + beta*feat_e + alpha*feat_p   restricted to columns of its half.
Inputs are pre-rolled on the host so every core runs an identical program
(its half is always columns 0:NH of its private x copy).

Channel-attention scores (x @ x.T over all N=4096 positions) are computed in a
3-pass bf16 hi/lo decomposition (hi*hi + hi*lo + lo*hi) so the transposed
operand can be produced with the 2-byte DMA xbar transpose; scores for the
position attention (fb/fc projections) are computed in fp32.  Value-side
matmuls run in bf16.  The 2*x term is computed exactly on the vector engine
from the fp32 input.
"""

import sys

sys.path.insert(0, "/opt/trn_rl_repo")

from contextlib import ExitStack

import numpy as np
import ml_dtypes

import concourse.bass as bass
import concourse.tile as tile
from concourse import bacc, mybir
from concourse.bass_utils import run_bass_kernel_spmd

F32 = mybir.dt.float32
BF16 = mybir.dt.bfloat16
AX = mybir.AxisListType
ALU = mybir.AluOpType
ACTF = mybir.ActivationFunctionType
BF = ml_dtypes.bfloat16

B, C, H, W = 4, 512, 64, 64
N = H * W            # 4096
NH = N // 2          # per-core position half
CP = C // 8          # 64 projection channels
N_CORES = 8


def _build_program(tc, ins, y_ap, C=C, N=N, NH=NH, CP=CP, lolo_pass=False):
    nc = tc.nc
    KT = C // 128          # channel k-tiles
    MT = N // 128          # position tiles (keys)
    CT = C // 128          # output channel tiles
    CHUNK = 512
    NCH = NH // CHUNK      # output column chunks

    x_f = ins["x"]

    ctx = ExitStack()
    sb = ctx.enter_context(tc.tile_pool(name="sb", bufs=1))
    ps = ctx.enter_context(tc.tile_pool(name="ps", bufs=1, space="PSUM"))

    def pst(shape, dtype=F32, name="pst"):
        return ps.tile(shape, dtype, tag="ps", bufs=8, name=name)

    # ---------------- constants / weights ----------------
    wcT = sb.tile([128, KT * CP], F32, name="wcT")
    nc.sync.dma_start(wcT[:].rearrange("p (kt m) -> p kt m", kt=KT),
                      ins["wcT"].rearrange("(kt p) m -> p kt m", p=128))
    wbT = sb.tile([128, KT * CP], F32, name="wbT")
    nc.sync.dma_start(wbT[:].rearrange("p (kt m) -> p kt m", kt=KT),
                      ins["wbT"].rearrange("(kt p) m -> p kt m", p=128))
    wdT = sb.tile([128, KT * C], BF16, name="wdT")
    nc.sync.dma_start(wdT[:].rearrange("p (kt m) -> p kt m", kt=KT),
                      ins["wdT"].rearrange("(kt p) m -> p kt m", p=128))
    bc_t = sb.tile([128, 1], F32, name="bc_t")
    nc.sync.dma_start(bc_t[:], ins["bc"])
    bb_t = sb.tile([128, 1], F32, name="bb_t")
    nc.sync.dma_start(bb_t[:], ins["bb"])
    bdrow = sb.tile([1, C], BF16, name="bdrow")
    nc.sync.dma_start(bdrow[:], ins["bdrow"])
    beta_t = sb.tile([128, 1], F32, name="beta_t")
    nc.sync.dma_start(beta_t[:], ins["beta"])
    alpha_t = sb.tile([1, 1], F32, name="alpha_t")
    nc.sync.dma_start(alpha_t[:], ins["alpha"])
    ident = sb.tile([128, 128], BF16, name="ident")
    nc.sync.dma_start(ident[:], ins["ident"])
    ones128 = sb.tile([128, 1], BF16, name="ones128")
    nc.sync.dma_start(ones128[:], ins["ones128"])
    onesrow_bf = sb.tile([1, 128], BF16, name="onesrow_bf")
    nc.sync.dma_start(onesrow_bf[:], ins["onesrow_bf"])
    onesrow_f32 = sb.tile([1, 128], F32, name="onesrow_f32")
    nc.sync.dma_start(onesrow_f32[:], ins["onesrow_f32"])

    x3 = x_f.rearrange("(kt p) n -> p kt n", p=128)  # [128, KT, N] DRAM view

    # ---------------- stage 1: fc (full), fb (first NH cols), fdT ----------------
    fc_t = sb.tile([64, N], F32, name="fc_t")
    fb_t = sb.tile([64, NH], F32, name="fb_t")
    fdT = sb.tile([128, MT * C], BF16, name="fdT")
    for ch in range(N // CHUNK):
        xs = sb.tile([128, KT * CHUNK], F32, tag="xs", bufs=2, name="xs")
        nc.sync.dma_start(xs[:].rearrange("p (kt n) -> p kt n", kt=KT),
                          x3[:, :, ch * CHUNK:(ch + 1) * CHUNK])
        xsb = sb.tile([128, KT * CHUNK], BF16, tag="xsb", bufs=2, name="xsb")
        nc.vector.tensor_copy(xsb[:], xs[:])
        ps_fc = pst([64, CHUNK], name="ps_fc")
        for kt in range(KT):
            nc.tensor.matmul(ps_fc[:], wcT[:, kt * CP:(kt + 1) * CP],
                             xs[:, kt * CHUNK:(kt + 1) * CHUNK],
                             start=(kt == 0), stop=(kt == KT - 1))
        nc.scalar.add(fc_t[:, ch * CHUNK:(ch + 1) * CHUNK], ps_fc[:], bc_t[0:64, :])
        if ch < NH // CHUNK:
            ps_fb = pst([64, CHUNK], name="ps_fb")
            for kt in range(KT):
                nc.tensor.matmul(ps_fb[:], wbT[:, kt * CP:(kt + 1) * CP],
                                 xs[:, kt * CHUNK:(kt + 1) * CHUNK],
                                 start=(kt == 0), stop=(kt == KT - 1))
            nc.scalar.add(fb_t[:, ch * CHUNK:(ch + 1) * CHUNK], ps_fb[:], bb_t[0:64, :])
        for j in range(CHUNK // 128):
            mt = ch * (CHUNK // 128) + j
            ps_d = pst([128, C], name="ps_d")
            for kt in range(KT):
                nc.tensor.matmul(ps_d[:], xsb[:, kt * CHUNK + j * 128: kt * CHUNK + (j + 1) * 128],
                                 wdT[:, kt * C:(kt + 1) * C],
                                 start=(kt == 0), stop=False)
            nc.tensor.matmul(ps_d[:], onesrow_bf[:], bdrow[:], start=False, stop=True)
            nc.scalar.copy(fdT[:, mt * C:(mt + 1) * C], ps_d[:])

    # ---------------- stage 3: channel attention scores (hi/lo passes) ----------------
    ps_att = [pst([128, C], name=f"ps_att{ct}") for ct in range(CT)]
    xhi3, xlo3 = ins["xhi"], ins["xlo"]
    for nt in range(MT):
        hiT = sb.tile([128, C], BF16, tag="hiT", bufs=3, name="hiT")
        nc.sync.dma_start_transpose(hiT[:], xhi3[:, nt * 128:(nt + 1) * 128])
        loT = sb.tile([128, C], BF16, tag="loT", bufs=3, name="loT")
        nc.sync.dma_start_transpose(loT[:], xlo3[:, nt * 128:(nt + 1) * 128])
        for ct in range(CT):
            cs = slice(ct * 128, (ct + 1) * 128)
            nc.tensor.matmul(ps_att[ct][:], hiT[:, cs], hiT[:, 0:C],
                             start=(nt == 0), stop=False)
            nc.tensor.matmul(ps_att[ct][:], hiT[:, cs], loT[:, 0:C],
                             start=False, stop=False)
            last = (nt == MT - 1) and not lolo_pass
            nc.tensor.matmul(ps_att[ct][:], loT[:, cs], hiT[:, 0:C],
                             start=False, stop=last)
            if lolo_pass:
                nc.tensor.matmul(ps_att[ct][:], loT[:, cs], loT[:, 0:C],
                                 start=False, stop=(nt == MT - 1))

    # ---------------- stage 3.5: channel softmax + transpose ----------------
    # softmax(rowmax - att) == exp(rowmin - att) / sum(exp(rowmin - att))
    rmin = sb.tile([128, CT], F32, name="rmin")
    attS = sb.tile([128, CT], F32, name="attS")
    recipc = sb.tile([128, CT], F32, name="recipc")
    attcT = sb.tile([128, KT * C], BF16, name="attcT")
    attc_tiles = []
    for ct in range(CT):
        nc.vector.tensor_reduce(rmin[:, ct:ct + 1], ps_att[ct][:], axis=AX.X, op=ALU.min)
        atte = sb.tile([128, C], F32, tag="atte", bufs=2, name="atte")
        nc.scalar.activation(atte[:], ps_att[ct][:], ACTF.Exp,
                             bias=rmin[:, ct:ct + 1], scale=-1.0,
                             accum_out=attS[:, ct:ct + 1])
        nc.vector.reciprocal(recipc[:, ct:ct + 1], attS[:, ct:ct + 1])
        attc = sb.tile([128, C], BF16, tag="attc", bufs=2, name="attc")
        nc.vector.tensor_scalar(attc[:], atte[:], recipc[:, ct:ct + 1], beta_t[:, 0:1],
                                op0=ALU.mult, op1=ALU.mult)
        attc_tiles.append(attc)
    for ct in range(CT):
        for dt in range(CT):
            ps_t = ps.tile([128, 128], BF16, tag="ps", bufs=8, name="ps_t")
            nc.tensor.transpose(ps_t[:], attc_tiles[ct][:, dt * 128:(dt + 1) * 128], ident[:])
            nc.scalar.copy(attcT[:, dt * C + ct * 128: dt * C + (ct + 1) * 128], ps_t[:])

    # ---------------- stage 4: position attention + combine, per 512-col chunk ----------------
    for ch in range(NCH):
        ncs = slice(ch * CHUNK, (ch + 1) * CHUNK)
        slab = sb.tile([128, MT * CHUNK], BF16, tag="slab", bufs=2, name="slab")
        ps_S = pst([1, CHUNK], name="ps_S")
        for mt in range(MT):
            ps_s = pst([128, CHUNK], name="ps_s")
            nc.tensor.matmul(ps_s[:], fc_t[0:64, mt * 128:(mt + 1) * 128],
                             fb_t[0:64, ncs], start=True, stop=True)
            nc.scalar.activation(slab[:, mt * CHUNK:(mt + 1) * CHUNK], ps_s[:], ACTF.Exp)
        for mt in range(MT):
            nc.tensor.matmul(ps_S[:], ones128[:], slab[:, mt * CHUNK:(mt + 1) * CHUNK],
                             start=(mt == 0), stop=(mt == MT - 1))
        recipS = sb.tile([1, CHUNK], F32, tag="recipS", bufs=1, name="recipS")
        nc.vector.reciprocal(recipS[:], ps_S[:])
        recipSa = sb.tile([1, CHUNK], F32, tag="recipSa", bufs=1, name="recipSa")
        nc.vector.tensor_scalar(recipSa[:], recipS[:], alpha_t[0:1, 0:1], None, op0=ALU.mult)
        ps_bc = pst([128, CHUNK], name="ps_bc")
        nc.tensor.matmul(ps_bc[:], onesrow_f32[:], recipSa[:], start=True, stop=True)
        bcast = sb.tile([128, CHUNK], F32, tag="bcast", bufs=2, name="bcast")
        nc.scalar.copy(bcast[:], ps_bc[:])

        x2_tiles, x2b_tiles = [], []
        for dt in range(CT):
            x2 = sb.tile([128, CHUNK], F32, tag="x2", bufs=5, name="x2")
            nc.sync.dma_start(x2[:], x_f[dt * 128:(dt + 1) * 128, ncs])
            x2b = sb.tile([128, CHUNK], BF16, tag="x2b", bufs=5, name="x2b")
            nc.vector.tensor_copy(x2b[:], x2[:])
            x2_tiles.append(x2)
            x2b_tiles.append(x2b)
        for ct in range(CT):
            ps_A = pst([128, CHUNK], name="ps_A")
            for mt in range(MT):
                nc.tensor.matmul(ps_A[:], fdT[:, mt * C + ct * 128: mt * C + (ct + 1) * 128],
                                 slab[:, mt * CHUNK:(mt + 1) * CHUNK],
                                 start=(mt == 0), stop=(mt == MT - 1))
            ps_C = pst([128, CHUNK], name="ps_C")
            for dt in range(KT):
                nc.tensor.matmul(ps_C[:], attcT[:, dt * C + ct * 128: dt * C + (ct + 1) * 128],
                                 x2b_tiles[dt][:],
                                 start=(dt == 0), stop=(dt == KT - 1))
            t1 = sb.tile([128, CHUNK], F32, tag="t1", bufs=2, name="t1")
            nc.vector.tensor_mul(t1[:], ps_A[:], bcast[:])
            t2 = sb.tile([128, CHUNK], F32, tag="t2", bufs=2, name="t2")
            nc.vector.scalar_tensor_tensor(t2[:], x2_tiles[ct][:], 2.0, ps_C[:],
                                           op0=ALU.mult, op1=ALU.add)
            outt = sb.tile([128, CHUNK], F32, tag="outt", bufs=3, name="outt")
            nc.vector.tensor_add(outt[:], t1[:], t2[:])
            nc.sync.dma_start(y_ap[ct * 128:(ct + 1) * 128, ncs], outt[:])

    ctx.close()


_CACHE = {}

_INPUT_SPECS = [
    ("x", [C, N], F32),
    ("xhi", [C, N], BF16),
    ("xlo", [C, N], BF16),
    ("wcT", [C, CP], F32),
    ("wbT", [C, CP], F32),
    ("wdT", [C, C], BF16),
    ("bc", [128, 1], F32),
    ("bb", [128, 1], F32),
    ("bdrow", [1, C], BF16),
    ("beta", [128, 1], F32),
    ("alpha", [1, 1], F32),
    ("ident", [128, 128], BF16),
    ("ones128", [128, 1], BF16),
    ("onesrow_bf", [1, 128], BF16),
    ("onesrow_f32", [1, 128], F32),
]


def get_compiled():
    if "nc" in _CACHE:
        return _CACHE["nc"]
    nc = bacc.Bacc("TRN2", target_bir_lowering=False, debug=False,
                   num_devices=N_CORES)
    ins = {}
    for name, shape, dt in _INPUT_SPECS:
        ins[name] = nc.dram_tensor(name, shape, dt, kind="ExternalInput").ap()
    y_ap = nc.dram_tensor("y", [C, NH], F32, kind="ExternalOutput").ap()
    with tile.TileContext(nc) as tc:
        _build_program(tc, ins, y_ap)
    nc.compile()
    _CACHE["nc"] = nc
    return nc


def make_in_maps(x, wb, bb, wc, bc, wd, bd, alpha, beta):
    """Build the 8 per-core input maps from the full problem inputs."""
    xb = np.ascontiguousarray(np.asarray(x, dtype=np.float32)).reshape(B, C, N)
    wb = np.asarray(wb, dtype=np.float32)
    wc = np.asarray(wc, dtype=np.float32)
    wd = np.asarray(wd, dtype=np.float32)
    bb_ = np.asarray(bb, dtype=np.float32).reshape(CP)
    bc_ = np.asarray(bc, dtype=np.float32).reshape(CP)
    bd_ = np.asarray(bd, dtype=np.float32).reshape(C)
    alpha_ = float(np.asarray(alpha).reshape(-1)[0])
    beta_ = float(np.asarray(beta).reshape(-1)[0])

    bc128 = np.zeros((128, 1), np.float32); bc128[:CP, 0] = bc_
    bb128 = np.zeros((128, 1), np.float32); bb128[:CP, 0] = bb_
    shared = {
        "wcT": np.ascontiguousarray(wc.T, dtype=np.float32),
        "wbT": np.ascontiguousarray(wb.T, dtype=np.float32),
        "wdT": np.ascontiguousarray(wd.T).astype(BF),
        "bc": bc128,
        "bb": bb128,
        "bdrow": bd_.reshape(1, C).astype(BF),
        "beta": np.full((128, 1), beta_, np.float32),
        "alpha": np.full((1, 1), alpha_, np.float32),
        "ident": np.eye(128, dtype=BF),
        "ones128": np.ones((128, 1), BF),
        "onesrow_bf": np.ones((1, 128), BF),
        "onesrow_f32": np.ones((1, 128), np.float32),
    }
    in_maps = []
    for core in range(N_CORES):
        b, h = core // 2, core % 2
        xc = xb[b] if h == 0 else np.ascontiguousarray(np.roll(xb[b], -NH, axis=1))
        # hi/lo split of the *unrolled* batch image: the channel-attention
        # score sums over all positions, so position order is irrelevant.
        key = ("hilo", b)
        if key not in _CACHE:
            xhi = xb[b].astype(BF)
            xlo = (xb[b] - xhi.astype(np.float32)).astype(BF)
            _CACHE[key] = (xhi, xlo)
        xhi, xlo = _CACHE[key]
        in_maps.append({"x": xc, "xhi": xhi, "xlo": xlo, **shared})
    return in_maps


def assemble_output(results):
    out = np.empty((B, C, N), np.float32)
    for core in range(N_CORES):
        b, h = core // 2, core % 2
        out[b][:, h * NH:(h + 1) * NH] = results[core]["y"]
    return out.reshape(B, C, H, W)


def kernel(x, wb, bb, wc, bc, wd, bd, alpha, beta):
    nc = get_compiled()
    in_maps = make_in_maps(x, wb, bb, wc, bc, wd, bd, alpha, beta)
    res = run_bass_kernel_spmd(nc, in_maps, list(range(N_CORES)))
    for key in [("hilo", b) for b in range(B)]:
        _CACHE.pop(key, None)
    return assemble_output(res.results)
